# revision 6
# baseline (speedup 1.0000x reference)
"""Boundary-map kernel for Trainium2, 8-core SPMD — raw Bacc, v7.

v13 over v12: job2/strip DVE order is DH, DV, H2, H4 and their PE groups
run wv4/wv2/wv2 (start) then w11 then wi (stop), so the 12-matmul wv block
starts as soon as DV lands instead of after H2 — pulls the tail in ~0.6us.

v12 over v9: no final s_out wait — the NEFF teardown barrier overlaps the
last output DMA's HBM-write receipt instead of serializing after it.

v9 over v7: job2's second-half threshold runs as is_gt on the idle DVE
(own semaphore s_a2), and the strip Sign no longer queues behind it on ACT.

v7 over v6:
  - x1 ships as two column-halves (D1a=[wt|x1-left] 392KB, D1b=[x1-right]),
    and job1's DH/H2 run as left/right pieces, so DVE starts ~1.5us earlier
    and PE's w11 group starts per-chunk as H2 pieces land
  - x2 / x2s / strip are three further DMAs; 6 input DMAs total
  - 10 warm matmuls (enough to flip HAM without delaying real work)

v6: output DMAs ship all 128 SBUF partitions (junk halo rows included,
sliced out on CPU) — full-partition SBUF->DRAM transfers spread across all
16 DMA engines (~305 GB/s) while partial ranges collapse to ~4 engines.

Math and layout otherwise as v2/v3 (see kernel_v2.py docstring).
"""

import numpy as np
import ml_dtypes

import concourse.bass as bass
import concourse.bacc as bacc
import concourse.mybir as mybir
from concourse import bass_utils

BF16 = mybir.dt.bfloat16
F32 = mybir.dt.float32
I8 = mybir.dt.int8
OP = mybir.AluOpType
AF = mybir.ActivationFunctionType

B, H, W = 2, 1024, 2048
NCORES = 8
C = 2052
CS = 516
RPC = 248
SROW = 899
XSPLIT = 1032          # x1 column split point (local)

WOFF = 0
X1, X1S = 512, 2564
X2, X2S = 4616, 6668
XS, XSS = 8720, 9236
BLOBC = 9752
D1A_END = 512 + XSPLIT   # 1544

W11, WI, WV4, WV2 = 0, 128, 256, 384

PROFILE = False
LAST_EXEC_NS = None
LAST_RESULTS = None


def _band(taps, P=128):
    w = np.zeros((P, P), np.float32)
    for m in range(P):
        for t in taps:
            k = m + t
            if 0 <= k < P:
                w[k, m] += 1.0
    return w.astype(ml_dtypes.bfloat16)


def make_weights():
    return np.concatenate(
        [_band([-1, 1]), _band([0]), _band([-2, -1, 0, 1]), _band([-1, 0])],
        axis=1,
    )


def build_nc():
    nc = bacc.Bacc("TRN2", target_bir_lowering=False, debug=False)
    xin = nc.dram_tensor("xin", [128, BLOBC], BF16, kind="ExternalInput").ap()
    y = nc.dram_tensor("y", [256, W], I8, kind="ExternalOutput").ap()
    ys = nc.dram_tensor("ys", [128, 512], I8, kind="ExternalOutput").ap()

    blob = nc.alloc_sbuf_tensor("blob", [128, BLOBC], BF16)
    tl = {}
    for s_ in ("a", "b"):
        for n_ in ("dh", "h2", "h4", "dv"):
            tl[n_ + s_] = nc.alloc_sbuf_tensor(n_ + s_, [128, C], BF16)
    for n_ in ("dh", "h2", "h4", "dv"):
        tl[n_ + "s"] = nc.alloc_sbuf_tensor(n_ + "s", [128, CS], BF16)
    e1 = nc.alloc_sbuf_tensor("e1", [128, W], I8)
    e2 = nc.alloc_sbuf_tensor("e2", [128, W], I8)
    es = nc.alloc_sbuf_tensor("es", [128, 512], I8)
    dw = nc.alloc_sbuf_tensor("dw", [128, 512], BF16)
    wsg = nc.alloc_sbuf_tensor("wsg", [128, 2], I8)

    pa = nc.alloc_psum_tensor("pa", [128, W], F32)
    pb = nc.alloc_psum_tensor("pb", [128, W], F32)

    s_in = [nc.alloc_semaphore(f"s_in{i}") for i in range(6)]
    s_v = nc.alloc_semaphore("s_v")
    s_t = nc.alloc_semaphore("s_t")
    s_a = nc.alloc_semaphore("s_a")
    s_a2 = nc.alloc_semaphore("s_a2")
    s_out = nc.alloc_semaphore("s_out")

    w11 = blob[:, WOFF + W11:WOFF + W11 + 128]
    wi = blob[:, WOFF + WI:WOFF + WI + 128]
    wv4 = blob[:, WOFF + WV4:WOFF + WV4 + 128]
    wv2 = blob[:, WOFF + WV2:WOFF + WV2 + 128]

    with nc.Block() as blk:

        @blk.sync
        def _(sync):
            for i, (a, b) in enumerate([(0, D1A_END), (D1A_END, X1S),
                                        (X1S, X2), (X2, X2S), (X2S, XS),
                                        (XS, BLOBC)]):
                sync.dma_start(blob[:, a:b], xin[:, a:b]).then_inc(s_in[i], 16)
            sync.wait_ge(s_a, 1)
            sync.dma_start(y[0:128, 0:1024], e1[:, 0:1024]).then_inc(s_out, 16)
            sync.wait_ge(s_a, 2)
            sync.dma_start(y[0:128, 1024:2048], e1[:, 1024:2048]).then_inc(s_out, 16)
            sync.wait_ge(s_a, 3)
            sync.dma_start(y[128:256, 0:1024], e2[:, 0:1024]).then_inc(s_out, 16)
            sync.wait_ge(s_a2, 1)
            sync.dma_start(y[128:256, 1024:2048], e2[:, 1024:2048]).then_inc(s_out, 16)
            sync.wait_ge(s_a, 4)
            sync.dma_start(ys[:, :], es[:, :]).then_inc(s_out, 16)

        @blk.vector
        def _(vector):
            nc.vector.memset(dw[:, :], 0.0).then_inc(s_v, 1)
            dh, h2, h4, dv = tl["dha"], tl["h2a"], tl["h4a"], tl["dva"]
            # job1, split into left/right column pieces
            vector.wait_ge(s_in[0], 16)
            nc.vector.tensor_tensor(   # DH-L -> v2
                out=dh[:, 0:XSPLIT - 1], in0=blob[:, X1:X1 + XSPLIT - 1],
                in1=blob[:, X1 + 1:X1 + XSPLIT], op=OP.not_equal).then_inc(s_v, 1)
            vector.wait_ge(s_in[1], 16)
            nc.vector.tensor_tensor(   # DH-R -> v3
                out=dh[:, XSPLIT - 1:C - 1], in0=blob[:, X1 + XSPLIT - 1:X1 + C - 1],
                in1=blob[:, X1 + XSPLIT:X1 + C], op=OP.not_equal).then_inc(s_v, 1)
            vector.wait_ge(s_v, 2)
            nc.vector.tensor_tensor(   # H2-L -> v4
                out=h2[:, 1:XSPLIT - 1], in0=dh[:, 0:XSPLIT - 2],
                in1=dh[:, 1:XSPLIT - 1], op=OP.add).then_inc(s_v, 1)
            vector.wait_ge(s_v, 3)
            nc.vector.tensor_tensor(   # H2-R -> v5
                out=h2[:, XSPLIT - 1:C - 1], in0=dh[:, XSPLIT - 2:C - 2],
                in1=dh[:, XSPLIT - 1:C - 1], op=OP.add).then_inc(s_v, 1)
            vector.wait_ge(s_in[2], 16)
            nc.vector.tensor_tensor(   # DV -> v6
                out=dv[:, 0:C], in0=blob[:, X1:X1 + C],
                in1=blob[:, X1S:X1S + C], op=OP.not_equal).then_inc(s_v, 1)
            vector.wait_ge(s_v, 5)
            nc.vector.tensor_tensor(   # H4 -> v7
                out=h4[:, 1:C - 3], in0=h2[:, 1:C - 3],
                in1=h2[:, 3:C - 1], op=OP.add).then_inc(s_v, 1)
            # job2 (vb=7): DH->8, H2->9, DV->10, H4->11
            # strip (vb=11): DH->12, H2->13, DV->14, H4->15
            for xo, xso, cc, sfx, sdh, sdv, vb in [
                (X2, X2S, C, "b", s_in[3], s_in[4], 7),
                (XS, XSS, CS, "s", s_in[5], s_in[5], 11),
            ]:
                dh, h2, h4, dv = (tl[n_ + sfx] for n_ in ("dh", "h2", "h4", "dv"))
                vector.wait_ge(sdh, 16)
                nc.vector.tensor_tensor(   # DH -> vb+1
                    out=dh[:, 0:cc - 1], in0=blob[:, xo:xo + cc - 1],
                    in1=blob[:, xo + 1:xo + cc], op=OP.not_equal).then_inc(s_v, 1)
                if sdv is not sdh:
                    vector.wait_ge(sdv, 16)
                nc.vector.tensor_tensor(   # DV -> vb+2
                    out=dv[:, 0:cc], in0=blob[:, xo:xo + cc],
                    in1=blob[:, xso:xso + cc], op=OP.not_equal).then_inc(s_v, 1)
                vector.wait_ge(s_v, vb + 1)
                nc.vector.tensor_tensor(   # H2 -> vb+3
                    out=h2[:, 1:cc - 1], in0=dh[:, 0:cc - 2],
                    in1=dh[:, 1:cc - 1], op=OP.add).then_inc(s_v, 1)
                vector.wait_ge(s_v, vb + 3)
                nc.vector.tensor_tensor(   # H4 -> vb+4
                    out=h4[:, 1:cc - 3], in0=h2[:, 1:cc - 3],
                    in1=h2[:, 3:cc - 1], op=OP.add).then_inc(s_v, 1)
            vector.wait_ge(s_t, 9)   # job2 chunks 2,3 accumulated
            nc.vector.tensor_scalar(
                out=e2[:, 1024:2048], in0=pb[:, 1024:2048],
                scalar1=0.0, scalar2=None, op0=OP.is_gt).then_inc(s_a2, 1)

        @blk.tensor
        def _(tensor):
            tensor.wait_ge(s_v, 1)
            for i in range(10):
                mm = nc.tensor.matmul(out=pb[:, 0:512], lhsT=dw[:, 0:128],
                                      rhs=dw[:, 0:512], start=True, stop=True)
                if i == 9:
                    mm.then_inc(s_t, 1)
            tensor.wait_ge(s_t, 1)
            tensor.wait_ge(s_in[0], 16)   # weights
            # job1: w11 split per H2 piece; t incs 2..5 on wi chunks
            dh, h2, h4, dv = tl["dha"], tl["h2a"], tl["h4a"], tl["dva"]
            for vneed, chunks in ((4, (0, 1)), (5, (2, 3))):
                tensor.wait_ge(s_v, vneed)
                for ci in chunks:
                    c0 = 512 * ci
                    nc.tensor.matmul(out=pa[:, c0:c0 + 512], lhsT=w11,
                                     rhs=h2[:, c0 + 2:c0 + 514],
                                     start=True, stop=False)
            tensor.wait_ge(s_v, 6)
            for wt, off in ((wv4, 2), (wv2, 1), (wv2, 3)):
                for ci in range(4):
                    c0 = 512 * ci
                    nc.tensor.matmul(out=pa[:, c0:c0 + 512], lhsT=wt,
                                     rhs=dv[:, c0 + off:c0 + off + 512],
                                     start=False, stop=False)
            tensor.wait_ge(s_v, 7)
            for ci in range(4):
                c0 = 512 * ci
                nc.tensor.matmul(out=pa[:, c0:c0 + 512], lhsT=wi,
                                 rhs=h4[:, c0 + 1:c0 + 513],
                                 start=False, stop=True).then_inc(s_t, 1)
            # job2 (t incs 6..9) and strip (t inc 10)
            for cc, sfx, ps, vb in ((C, "b", pb, 7), (CS, "s", pa, 11)):
                dh, h2, h4, dv = (tl[n_ + sfx] for n_ in ("dh", "h2", "h4", "dv"))
                nchunk = (cc - 4) // 512
                if sfx == "s":
                    tensor.wait_ge(s_a, 1)
                groups = [
                    (wv4, dv, 2, vb + 2, True, False),
                    (wv2, dv, 1, vb + 2, False, False),
                    (wv2, dv, 3, vb + 2, False, False),
                    (w11, h2, 2, vb + 3, False, False),
                    (wi, h4, 1, vb + 4, False, True),
                ]
                for wt, src, off, vneed, st, sp in groups:
                    tensor.wait_ge(s_v, vneed)
                    for ci in range(nchunk):
                        c0 = 512 * ci
                        mm = nc.tensor.matmul(
                            out=ps[:, c0:c0 + 512], lhsT=wt,
                            rhs=src[:, c0 + off:c0 + off + 512],
                            start=st, stop=sp)
                        if sp:
                            mm.then_inc(s_t, 1)

        @blk.scalar
        def _(scalar):
            scalar.wait_ge(s_v, 1)
            nc.scalar.activation(out=wsg[:, :], in_=dw[:, 0:2], func=AF.Sign)
            scalar.wait_ge(s_t, 3)
            nc.scalar.activation(out=e1[:, 0:1024], in_=pa[:, 0:1024], func=AF.Sign).then_inc(s_a, 1)
            scalar.wait_ge(s_t, 5)
            nc.scalar.activation(out=e1[:, 1024:2048], in_=pa[:, 1024:2048], func=AF.Sign).then_inc(s_a, 1)
            scalar.wait_ge(s_t, 7)
            nc.scalar.activation(out=e2[:, 0:1024], in_=pb[:, 0:1024], func=AF.Sign).then_inc(s_a, 1)
            scalar.wait_ge(s_t, 10)
            nc.scalar.activation(out=es[:, :], in_=pa[:, 0:512], func=AF.Sign).then_inc(s_a, 1)

    nc.compile()
    return nc


def make_in_maps(gtmasks):
    lab = np.asarray(gtmasks)[:, 0]
    wcat = make_weights()
    padded = [
        np.pad(lab[b], ((2, 2), (2, 2))).astype(ml_dtypes.bfloat16)
        for b in range(B)
    ]
    in_maps = []
    for c in range(NCORES):
        b, q = divmod(c, 4)
        pf = padded[b]
        r0 = RPC * q
        xin = np.empty((128, BLOBC), dtype=ml_dtypes.bfloat16)
        xin[:, WOFF:WOFF + 512] = wcat
        xin[:, X1:X1 + C] = pf[r0:r0 + 128]
        xin[:, X1S:X1S + C] = pf[r0 + 1:r0 + 129]
        xin[:, X2:X2 + C] = pf[r0 + 124:r0 + 252]
        xin[:, X2S:X2S + C] = pf[r0 + 125:r0 + 253]
        xin[:, XS:XS + CS] = pf[SROW:SROW + 128, 512 * q:512 * q + CS]
        xin[:, XSS:XSS + CS] = pf[SROW + 1:SROW + 129, 512 * q:512 * q + CS]
        in_maps.append({"xin": np.ascontiguousarray(xin)})
    return in_maps


def assemble(results):
    out = np.zeros((B, 1, H, W), np.int32)
    for c in range(NCORES):
        b, q = divmod(c, 4)
        yv = results[c]["y"]
        out[b, 0, RPC * q:RPC * q + 124, :] = yv[2:126, :]
        out[b, 0, RPC * q + 124:RPC * (q + 1), :] = yv[130:254, :]
        out[b, 0, H - 32:, 512 * q:512 * (q + 1)] = results[c]["ys"][95:127, :]
    return out


def kernel(gtmasks):
    global LAST_EXEC_NS, LAST_RESULTS
    in_maps = make_in_maps(gtmasks)
    nc = build_nc()
    # untraced warm-up execution: the first run after compile/idle lands in a
    # slow device phase (cold HBM/clock state); discard it and measure steady
    bass_utils.run_bass_kernel_spmd(
        nc, in_maps, core_ids=list(range(NCORES)), trace=False)
    res = bass_utils.run_bass_kernel_spmd(
        nc, in_maps, core_ids=list(range(NCORES)), trace=PROFILE)
    LAST_EXEC_NS = res.exec_time_ns
    LAST_RESULTS = res
    return assemble(res.results)


# revision 7
# speedup vs baseline: 1.1829x; 1.1829x over previous
"""Boundary-map kernel for Trainium2, 8-core SPMD — raw Bacc, v7.

v13 over v12: job2/strip DVE order is DH, DV, H2, H4 and their PE groups
run wv4/wv2/wv2 (start) then w11 then wi (stop), so the 12-matmul wv block
starts as soon as DV lands instead of after H2 — pulls the tail in ~0.6us.

v12 over v9: no final s_out wait — the NEFF teardown barrier overlaps the
last output DMA's HBM-write receipt instead of serializing after it.

v9 over v7: job2's second-half threshold runs as is_gt on the idle DVE
(own semaphore s_a2), and the strip Sign no longer queues behind it on ACT.

v7 over v6:
  - x1 ships as two column-halves (D1a=[wt|x1-left] 392KB, D1b=[x1-right]),
    and job1's DH/H2 run as left/right pieces, so DVE starts ~1.5us earlier
    and PE's w11 group starts per-chunk as H2 pieces land
  - x2 / x2s / strip are three further DMAs; 6 input DMAs total
  - 10 warm matmuls (enough to flip HAM without delaying real work)

v6: output DMAs ship all 128 SBUF partitions (junk halo rows included,
sliced out on CPU) — full-partition SBUF->DRAM transfers spread across all
16 DMA engines (~305 GB/s) while partial ranges collapse to ~4 engines.

Math and layout otherwise as v2/v3 (see kernel_v2.py docstring).
"""

import numpy as np
import ml_dtypes

import concourse.bass as bass
import concourse.bacc as bacc
import concourse.mybir as mybir
from concourse import bass_utils

BF16 = mybir.dt.bfloat16
F32 = mybir.dt.float32
I8 = mybir.dt.int8
OP = mybir.AluOpType
AF = mybir.ActivationFunctionType

B, H, W = 2, 1024, 2048
NCORES = 8
C = 2052
CS = 516
RPC = 248
SROW = 899
XSPLIT = 1032          # x1 column split point (local)

WOFF = 0
X1, X1S = 512, 2564
X2, X2S = 4616, 6668
XS, XSS = 8720, 9236
BLOBC = 9752
D1A_END = 512 + XSPLIT   # 1544

W11, WI, WV4, WV2 = 0, 128, 256, 384

PROFILE = False
LAST_EXEC_NS = None
LAST_RESULTS = None


def _band(taps, P=128):
    w = np.zeros((P, P), np.float32)
    for m in range(P):
        for t in taps:
            k = m + t
            if 0 <= k < P:
                w[k, m] += 1.0
    return w.astype(ml_dtypes.bfloat16)


def make_weights():
    return np.concatenate(
        [_band([-1, 1]), _band([0]), _band([-2, -1, 0, 1]), _band([-1, 0])],
        axis=1,
    )


def build_nc():
    nc = bacc.Bacc("TRN2", target_bir_lowering=False, debug=False)
    xin = nc.dram_tensor("xin", [128, BLOBC], BF16, kind="ExternalInput").ap()
    y = nc.dram_tensor("y", [256, W], I8, kind="ExternalOutput").ap()
    ys = nc.dram_tensor("ys", [128, 512], I8, kind="ExternalOutput").ap()

    blob = nc.alloc_sbuf_tensor("blob", [128, BLOBC], BF16)
    tl = {}
    for s_ in ("a", "b"):
        for n_ in ("dh", "h2", "h4", "dv"):
            tl[n_ + s_] = nc.alloc_sbuf_tensor(n_ + s_, [128, C], BF16)
    for n_ in ("dh", "h2", "h4", "dv"):
        tl[n_ + "s"] = nc.alloc_sbuf_tensor(n_ + "s", [128, CS], BF16)
    e1 = nc.alloc_sbuf_tensor("e1", [128, W], I8)
    e2 = nc.alloc_sbuf_tensor("e2", [128, W], I8)
    es = nc.alloc_sbuf_tensor("es", [128, 512], I8)
    dw = nc.alloc_sbuf_tensor("dw", [128, 512], BF16)
    wsg = nc.alloc_sbuf_tensor("wsg", [128, 2], I8)

    pa = nc.alloc_psum_tensor("pa", [128, W], F32)
    pb = nc.alloc_psum_tensor("pb", [128, W], F32)

    s_in = [nc.alloc_semaphore(f"s_in{i}") for i in range(6)]
    s_v = nc.alloc_semaphore("s_v")
    s_t = nc.alloc_semaphore("s_t")
    s_a = nc.alloc_semaphore("s_a")
    s_a2 = nc.alloc_semaphore("s_a2")
    s_out = nc.alloc_semaphore("s_out")

    w11 = blob[:, WOFF + W11:WOFF + W11 + 128]
    wi = blob[:, WOFF + WI:WOFF + WI + 128]
    wv4 = blob[:, WOFF + WV4:WOFF + WV4 + 128]
    wv2 = blob[:, WOFF + WV2:WOFF + WV2 + 128]

    with nc.Block() as blk:

        @blk.sync
        def _(sync):
            for i, (a, b) in enumerate([(0, D1A_END), (D1A_END, X1S),
                                        (X1S, X2), (X2, X2S), (X2S, XS),
                                        (XS, BLOBC)]):
                sync.dma_start(blob[:, a:b], xin[:, a:b]).then_inc(s_in[i], 16)
            sync.wait_ge(s_a, 1)
            sync.dma_start(y[0:128, 0:1024], e1[:, 0:1024]).then_inc(s_out, 16)
            sync.wait_ge(s_a, 2)
            sync.dma_start(y[0:128, 1024:2048], e1[:, 1024:2048]).then_inc(s_out, 16)
            sync.wait_ge(s_a, 3)
            sync.dma_start(y[128:256, 0:1024], e2[:, 0:1024]).then_inc(s_out, 16)
            sync.wait_ge(s_a2, 1)
            sync.dma_start(y[128:256, 1024:2048], e2[:, 1024:2048]).then_inc(s_out, 16)
            sync.wait_ge(s_a, 4)
            sync.dma_start(ys[:, :], es[:, :]).then_inc(s_out, 16)

        @blk.vector
        def _(vector):
            nc.vector.memset(dw[:, :], 0.0).then_inc(s_v, 1)
            dh, h2, h4, dv = tl["dha"], tl["h2a"], tl["h4a"], tl["dva"]
            # job1, split into left/right column pieces
            vector.wait_ge(s_in[0], 16)
            nc.vector.tensor_tensor(   # DH-L -> v2
                out=dh[:, 0:XSPLIT - 1], in0=blob[:, X1:X1 + XSPLIT - 1],
                in1=blob[:, X1 + 1:X1 + XSPLIT], op=OP.not_equal).then_inc(s_v, 1)
            vector.wait_ge(s_in[1], 16)
            nc.vector.tensor_tensor(   # DH-R -> v3
                out=dh[:, XSPLIT - 1:C - 1], in0=blob[:, X1 + XSPLIT - 1:X1 + C - 1],
                in1=blob[:, X1 + XSPLIT:X1 + C], op=OP.not_equal).then_inc(s_v, 1)
            vector.wait_ge(s_v, 2)
            nc.vector.tensor_tensor(   # H2-L -> v4
                out=h2[:, 1:XSPLIT - 1], in0=dh[:, 0:XSPLIT - 2],
                in1=dh[:, 1:XSPLIT - 1], op=OP.add).then_inc(s_v, 1)
            vector.wait_ge(s_v, 3)
            nc.vector.tensor_tensor(   # H2-R -> v5
                out=h2[:, XSPLIT - 1:C - 1], in0=dh[:, XSPLIT - 2:C - 2],
                in1=dh[:, XSPLIT - 1:C - 1], op=OP.add).then_inc(s_v, 1)
            vector.wait_ge(s_in[2], 16)
            nc.vector.tensor_tensor(   # DV -> v6
                out=dv[:, 0:C], in0=blob[:, X1:X1 + C],
                in1=blob[:, X1S:X1S + C], op=OP.not_equal).then_inc(s_v, 1)
            vector.wait_ge(s_v, 5)
            nc.vector.tensor_tensor(   # H4 -> v7
                out=h4[:, 1:C - 3], in0=h2[:, 1:C - 3],
                in1=h2[:, 3:C - 1], op=OP.add).then_inc(s_v, 1)
            # job2 (vb=7): DH->8, H2->9, DV->10, H4->11
            # strip (vb=11): DH->12, H2->13, DV->14, H4->15
            for xo, xso, cc, sfx, sdh, sdv, vb in [
                (X2, X2S, C, "b", s_in[3], s_in[4], 7),
                (XS, XSS, CS, "s", s_in[5], s_in[5], 11),
            ]:
                dh, h2, h4, dv = (tl[n_ + sfx] for n_ in ("dh", "h2", "h4", "dv"))
                vector.wait_ge(sdh, 16)
                nc.vector.tensor_tensor(   # DH -> vb+1
                    out=dh[:, 0:cc - 1], in0=blob[:, xo:xo + cc - 1],
                    in1=blob[:, xo + 1:xo + cc], op=OP.not_equal).then_inc(s_v, 1)
                if sdv is not sdh:
                    vector.wait_ge(sdv, 16)
                nc.vector.tensor_tensor(   # DV -> vb+2
                    out=dv[:, 0:cc], in0=blob[:, xo:xo + cc],
                    in1=blob[:, xso:xso + cc], op=OP.not_equal).then_inc(s_v, 1)
                vector.wait_ge(s_v, vb + 1)
                nc.vector.tensor_tensor(   # H2 -> vb+3
                    out=h2[:, 1:cc - 1], in0=dh[:, 0:cc - 2],
                    in1=dh[:, 1:cc - 1], op=OP.add).then_inc(s_v, 1)
                vector.wait_ge(s_v, vb + 3)
                nc.vector.tensor_tensor(   # H4 -> vb+4
                    out=h4[:, 1:cc - 3], in0=h2[:, 1:cc - 3],
                    in1=h2[:, 3:cc - 1], op=OP.add).then_inc(s_v, 1)
            vector.wait_ge(s_t, 9)   # job2 chunks 2,3 accumulated
            nc.vector.tensor_scalar(
                out=e2[:, 1024:2048], in0=pb[:, 1024:2048],
                scalar1=0.0, scalar2=None, op0=OP.is_gt).then_inc(s_a2, 1)

        @blk.tensor
        def _(tensor):
            tensor.wait_ge(s_v, 1)
            for i in range(10):
                mm = nc.tensor.matmul(out=pb[:, 0:512], lhsT=dw[:, 0:128],
                                      rhs=dw[:, 0:512], start=True, stop=True)
                if i == 9:
                    mm.then_inc(s_t, 1)
            tensor.wait_ge(s_t, 1)
            tensor.wait_ge(s_in[0], 16)   # weights
            # job1: w11 split per H2 piece; t incs 2..5 on wi chunks
            dh, h2, h4, dv = tl["dha"], tl["h2a"], tl["h4a"], tl["dva"]
            for vneed, chunks in ((4, (0, 1)), (5, (2, 3))):
                tensor.wait_ge(s_v, vneed)
                for ci in chunks:
                    c0 = 512 * ci
                    nc.tensor.matmul(out=pa[:, c0:c0 + 512], lhsT=w11,
                                     rhs=h2[:, c0 + 2:c0 + 514],
                                     start=True, stop=False)
            tensor.wait_ge(s_v, 6)
            for wt, off in ((wv4, 2), (wv2, 1), (wv2, 3)):
                for ci in range(4):
                    c0 = 512 * ci
                    nc.tensor.matmul(out=pa[:, c0:c0 + 512], lhsT=wt,
                                     rhs=dv[:, c0 + off:c0 + off + 512],
                                     start=False, stop=False)
            tensor.wait_ge(s_v, 7)
            for ci in range(4):
                c0 = 512 * ci
                nc.tensor.matmul(out=pa[:, c0:c0 + 512], lhsT=wi,
                                 rhs=h4[:, c0 + 1:c0 + 513],
                                 start=False, stop=True).then_inc(s_t, 1)
            # job2 (t incs 6..9) and strip (t inc 10)
            for cc, sfx, ps, vb in ((C, "b", pb, 7), (CS, "s", pa, 11)):
                dh, h2, h4, dv = (tl[n_ + sfx] for n_ in ("dh", "h2", "h4", "dv"))
                nchunk = (cc - 4) // 512
                if sfx == "s":
                    tensor.wait_ge(s_a, 1)
                groups = [
                    (wv4, dv, 2, vb + 2, True, False),
                    (wv2, dv, 1, vb + 2, False, False),
                    (wv2, dv, 3, vb + 2, False, False),
                    (w11, h2, 2, vb + 3, False, False),
                    (wi, h4, 1, vb + 4, False, True),
                ]
                for wt, src, off, vneed, st, sp in groups:
                    tensor.wait_ge(s_v, vneed)
                    for ci in range(nchunk):
                        c0 = 512 * ci
                        mm = nc.tensor.matmul(
                            out=ps[:, c0:c0 + 512], lhsT=wt,
                            rhs=src[:, c0 + off:c0 + off + 512],
                            start=st, stop=sp)
                        if sp:
                            mm.then_inc(s_t, 1)

        @blk.scalar
        def _(scalar):
            scalar.wait_ge(s_v, 1)
            nc.scalar.activation(out=wsg[:, :], in_=dw[:, 0:2], func=AF.Sign)
            scalar.wait_ge(s_t, 3)
            nc.scalar.activation(out=e1[:, 0:1024], in_=pa[:, 0:1024], func=AF.Sign).then_inc(s_a, 1)
            scalar.wait_ge(s_t, 5)
            nc.scalar.activation(out=e1[:, 1024:2048], in_=pa[:, 1024:2048], func=AF.Sign).then_inc(s_a, 1)
            scalar.wait_ge(s_t, 7)
            nc.scalar.activation(out=e2[:, 0:1024], in_=pb[:, 0:1024], func=AF.Sign).then_inc(s_a, 1)
            scalar.wait_ge(s_t, 10)
            nc.scalar.activation(out=es[:, :], in_=pa[:, 0:512], func=AF.Sign).then_inc(s_a, 1)

    nc.compile()
    return nc


def make_in_maps(gtmasks):
    lab = np.asarray(gtmasks)[:, 0]
    wcat = make_weights()
    padded = [
        np.pad(lab[b], ((2, 2), (2, 2))).astype(ml_dtypes.bfloat16)
        for b in range(B)
    ]
    in_maps = []
    for c in range(NCORES):
        b, q = divmod(c, 4)
        pf = padded[b]
        r0 = RPC * q
        xin = np.empty((128, BLOBC), dtype=ml_dtypes.bfloat16)
        xin[:, WOFF:WOFF + 512] = wcat
        xin[:, X1:X1 + C] = pf[r0:r0 + 128]
        xin[:, X1S:X1S + C] = pf[r0 + 1:r0 + 129]
        xin[:, X2:X2 + C] = pf[r0 + 124:r0 + 252]
        xin[:, X2S:X2S + C] = pf[r0 + 125:r0 + 253]
        xin[:, XS:XS + CS] = pf[SROW:SROW + 128, 512 * q:512 * q + CS]
        xin[:, XSS:XSS + CS] = pf[SROW + 1:SROW + 129, 512 * q:512 * q + CS]
        in_maps.append({"xin": np.ascontiguousarray(xin)})
    return in_maps


def assemble(results):
    out = np.zeros((B, 1, H, W), np.int32)
    for c in range(NCORES):
        b, q = divmod(c, 4)
        yv = results[c]["y"]
        out[b, 0, RPC * q:RPC * q + 124, :] = yv[2:126, :]
        out[b, 0, RPC * q + 124:RPC * (q + 1), :] = yv[130:254, :]
        out[b, 0, H - 32:, 512 * q:512 * (q + 1)] = results[c]["ys"][95:127, :]
    return out


def kernel(gtmasks):
    global LAST_EXEC_NS, LAST_RESULTS
    in_maps = make_in_maps(gtmasks)
    nc = build_nc()
    res = bass_utils.run_bass_kernel_spmd(
        nc, in_maps, core_ids=list(range(NCORES)), trace=PROFILE)
    LAST_EXEC_NS = res.exec_time_ns
    LAST_RESULTS = res
    return assemble(res.results)
